# revision 7
# baseline (speedup 1.0000x reference)
"""Trainium2 Bass kernel for windowed 3D attention (sparse_attention).

Reference computation (per window of the 16x16 grid, 256 windows total):
  tokens N = 6*7*7 = 294, d=256, 8 heads of 32
  qkv = x @ w_qkv.T ; per head: A = softmax(q k^T/sqrt(dh) + bias) ; out = (A v) @ w_out.T

Sharding: data-parallel over the window grid; core s takes X-rows [2s, 2s+2)
(32 windows). Params + bias table replicated.

Device layout strategy (all matmuls keep the contraction dim on partitions):
  - host pre-transposes x to d-major xT [256, 9408] per core
  - Q^T/K^T computed d-major (lhsT = w_qkv^T), V computed token-major
  - S^T[j,i] per head via lhsT=K_h^T (row-packed 4 heads, K=32)
  - A^T = exp(S^T) * exp(bias)^T  (exp on ScalarE from PSUM, bias-mult on DVE/GPSIMD
    with host-precomputed exp(bias), bf16)
  - rowsum via col-packed ones-matmuls, transposed on PE to [i, heads] layout
  - O^T = V^T A^T col-packed; transpose on PE -> normalize per (head,i) -> transpose back
  - Y = O_norm @ w_out^T token-major, DMA'd out contiguously
"""

import os
from contextlib import ExitStack

import numpy as np
import ml_dtypes

import concourse.bass as bass
import concourse.mybir as mybir
import concourse.tile as tile
from concourse import bacc
from concourse.bass_utils import run_bass_kernel_spmd
from concourse.masks import make_identity

F32 = mybir.dt.float32
F32R = mybir.dt.float32r
BF16 = mybir.dt.bfloat16

L, W, D, H = 6, 7, 256, 8
DH = D // H                      # 32
N = L * W * W                    # 294
GX = GY = 16
NCORES = 8
XPC = GX // NCORES               # X-rows per core
NW = XPC * GY                    # 32 windows per core
TOK = NW * N                     # 9408 tokens per core
SCALE = DH ** -0.5

JCH = [(0, 128), (128, 128), (256, 38)]   # j chunks (contraction / partition dim)
ICH = [(0, 98), (98, 98), (196, 98)]      # i chunks (query tokens; 98 = 2 l-blocks)

TRACE = False     # set by test.py for profiling runs
_CACHE = {}


def _r(ap, dt=F32R):
    return ap.bitcast(dt)


def _bcast_free(ap, count):
    """Append a stride-0 innermost free dim of size `count` to an AP."""
    return bass.AP(tensor=ap.tensor, offset=ap.offset, ap=list(ap.ap) + [[0, count]])


def _body(ctx, tc, xT, wqkvT, woutT, expBT, y):
    nc = tc.nc

    const = ctx.enter_context(tc.tile_pool(name="const", bufs=1))
    xpool = ctx.enter_context(tc.tile_pool(name="xin", bufs=3))
    qkpool = ctx.enter_context(tc.tile_pool(name="qk", bufs=2))
    vpool = ctx.enter_context(tc.tile_pool(name="vtok", bufs=2))
    epool = ctx.enter_context(tc.tile_pool(name="exptmp", bufs=3))
    apool = ctx.enter_context(tc.tile_pool(name="at", bufs=7))
    opool = ctx.enter_context(tc.tile_pool(name="ot", bufs=2))
    onpool = ctx.enter_context(tc.tile_pool(name="onorm", bufs=2))
    o2pool = ctx.enter_context(tc.tile_pool(name="ot2", bufs=2))
    ypool = ctx.enter_context(tc.tile_pool(name="yout", bufs=3))
    rspool = ctx.enter_context(tc.tile_pool(name="rs", bufs=2))
    rcpool = ctx.enter_context(tc.tile_pool(name="recip", bufs=2))

    ps_s = ctx.enter_context(tc.tile_pool(name="ps_s", bufs=1, space="PSUM"))
    ps_o = ctx.enter_context(tc.tile_pool(name="ps_o", bufs=2, space="PSUM"))
    ps_m = ctx.enter_context(tc.tile_pool(name="ps_m", bufs=2, space="PSUM"))

    # ---- resident constants ----
    wqkv_r = wqkvT.rearrange("(c p) n -> p c n", c=2)     # [128, 2, 768]
    wqkv_s = const.tile([128, 2, 2 * D], F32)     # Q^T,K^T weight cols only
    nc.sync.dma_start(out=_r(wqkv_s), in_=_r(wqkv_r[:, :, 0:2 * D]))
    wv_s = const.tile([128, 2, D], F32)
    nc.sync.dma_start(out=_r(wv_s), in_=_r(wqkv_r[:, :, 2 * D:3 * D]))
    wout_s = const.tile([128, 2, D], F32)
    nc.sync.dma_start(out=_r(wout_s), in_=_r(woutT.rearrange("(c p) n -> p c n", c=2)))
    expb_s = const.tile([128, 3, H * N], BF16)
    for jc, (j0, jn) in enumerate(JCH):
        nc.sync.dma_start(out=expb_s[:jn, jc, :], in_=expBT[j0:j0 + jn, :])
    ident = const.tile([128, 128], F32)
    make_identity(nc, ident)
    ones_b = const.tile([128, 1], BF16)
    nc.vector.memset(ones_b, 1.0)

    for w in range(NW):
        t0 = w * N
        # ---- load x window (d-major) ----
        xw = xpool.tile([128, 2, N], F32)
        nc.sync.dma_start(out=_r(xw), in_=_r(xT.rearrange("(c p) t -> p c t", c=2)[:, :, t0:t0 + N]))

        # ---- Q^T / K^T (d-major, scaled q) ----
        qk = qkpool.tile([128, 4, N], BF16)      # m=0,1: q^T*scale ; m=2,3: k^T
        for m in range(4):
            pq = ps_m.tile([128, 512], F32, tag="psmisc")
            for kc in range(2):
                nc.tensor.matmul(
                    pq[:, :N], _r(wqkv_s[:, kc, m * 128:(m + 1) * 128]), _r(xw[:, kc, :]),
                    start=(kc == 0), stop=(kc == 1))
            if m < 2:
                nc.scalar.activation(qk[:, m, :], pq[:, :N],
                                     mybir.ActivationFunctionType.Copy, scale=float(SCALE))
            else:
                nc.vector.tensor_copy(qk[:, m, :], pq[:, :N])

        # ---- V token-major ----
        vtok = vpool.tile([128, 3, D], BF16)
        for jc, (j0, jn) in enumerate(JCH):
            pv = ps_m.tile([128, 512], F32, tag="psmisc")
            for kc in range(2):
                nc.tensor.matmul(
                    pv[:jn, :D], _r(xw[:, kc, j0:j0 + jn]), _r(wv_s[:, kc, :]),
                    start=(kc == 0), stop=(kc == 1))
            nc.vector.tensor_copy(vtok[:jn, jc, :], pv[:jn, :D])

        # ---- S^T = K^T.T @ Q^T  (row-packed 4 heads) + exp + bias-mult ----
        at = []
        for jc, (j0, jn) in enumerate(JCH):
            a_t = apool.tile([128, H * N], BF16, tag="at")
            for g in range(2):
                ps4 = ps_s.tile([128, 4, 512], F32, tag="s4")
                for c in range(4):
                    h = 4 * g + c
                    nc.tensor.matmul(
                        ps4[:jn, c, :N],
                        qk[32 * c:32 * c + 32, 2 + g, j0:j0 + jn],
                        qk[32 * c:32 * c + 32, g, :],
                        start=True, stop=True, tile_position=(32 * c, 0))
                etmp = epool.tile([128, 4 * N], BF16, tag="etmp")
                nc.scalar.activation(
                    etmp[:jn, :].rearrange("p (c i) -> p c i", c=4),
                    ps4[:jn, :, :N], mybir.ActivationFunctionType.Exp)
                eng = nc.vector if g == 0 else nc.gpsimd
                eng.tensor_tensor(
                    out=a_t[:jn, g * 4 * N:(g + 1) * 4 * N],
                    in0=etmp[:jn, :],
                    in1=expb_s[:jn, jc, g * 4 * N:(g + 1) * 4 * N],
                    op=mybir.AluOpType.mult)
            at.append(a_t)

        # ---- rowsums (col-packed ones matmuls, accumulated over j chunks) ----
        prs = [ps_m.tile([128, 512], F32, tag="psmisc", name=f"prs{g}") for g in range(2)]
        for jc, (j0, jn) in enumerate(JCH):
            for g in range(2):
                for c in range(4):
                    h = 4 * g + c
                    nc.tensor.matmul(
                        prs[g][32 * c:32 * c + 1, :N],
                        ones_b[:jn, :], at[jc][:jn, h * N:(h + 1) * N],
                        start=(jc == 0), stop=(jc == 2),
                        tile_position=(0, 32 * c), skip_group_check=True)
        rs_s = rspool.tile([128, 2, N], F32)
        for g in range(2):
            nc.vector.tensor_copy(rs_s[:, g, :], prs[g][:, :N])
        # transpose rowsums to [i, head] and take reciprocals
        rcp = rcpool.tile([128, 3, 8], F32)
        for ic, (i0, isz) in enumerate(ICH):
            prt = ps_m.tile([128, 2, 128], F32, tag="psmisc")
            for g in range(2):
                nc.tensor.transpose(prt[:isz, g, :], rs_s[:, g, i0:i0 + isz], ident)
            for g in range(2):
                src = prt[:isz, g, :].rearrange("p (c r) -> p c r", r=32)[:, :, 0]
                nc.vector.reciprocal(rcp[:isz, ic, 4 * g:4 * g + 4], src)

        # ---- O^T = V^T A^T (col-packed 4 heads) ----
        po = [ps_o.tile([128, 512], F32, tag="po", name=f"po{g}") for g in range(2)]
        for jc, (j0, jn) in enumerate(JCH):
            for g in range(2):
                for c in range(4):
                    h = 4 * g + c
                    nc.tensor.matmul(
                        po[g][32 * c:32 * c + 32, :N],
                        vtok[:jn, jc, 32 * h:32 * h + 32],
                        at[jc][:jn, h * N:(h + 1) * N],
                        start=(jc == 0), stop=(jc == 2),
                        tile_position=(0, 32 * c), skip_group_check=True)
        ot = opool.tile([128, 2, N], F32)
        for g in range(2):
            nc.vector.tensor_copy(ot[:, g, :], po[g][:, :N])

        # ---- transpose O^T -> O, normalize per (head, i) ----
        onrm = onpool.tile([128, 3, D], F32)
        for ic, (i0, isz) in enumerate(ICH):
            pot = ps_m.tile([128, 2, 128], F32, tag="psmisc")
            for g in range(2):
                nc.tensor.transpose(pot[:isz, g, :], ot[:, g, i0:i0 + isz], ident)
            nc.vector.tensor_tensor(
                out=onrm[:isz, ic, :].rearrange("p (h e) -> p h e", h=8),
                in0=pot[:isz, :, :],
                in1=_bcast_free(rcp[:isz, ic, :], 32),
                op=mybir.AluOpType.mult)

        # ---- transpose back: O_norm -> O_norm^T ----
        po2 = [ps_o.tile([128, 512], F32, tag="po", name=f"po2_{dc}") for dc in range(2)]
        for ic, (i0, isz) in enumerate(ICH):
            for dc in range(2):
                nc.tensor.transpose(
                    po2[dc][:, i0:i0 + isz],
                    onrm[:isz, ic, dc * 128:(dc + 1) * 128], ident[:isz, :isz])
        ot2 = o2pool.tile([128, 2, N], F32)
        for dc in range(2):
            nc.vector.tensor_copy(_r(ot2[:, dc, :]), po2[dc][:, :N])

        # ---- Y = O_norm @ w_out^T (token-major) + store ----
        ysb = ypool.tile([128, 3, D], F32)
        for ic, (i0, isz) in enumerate(ICH):
            py = ps_m.tile([128, 512], F32, tag="psmisc")
            for dc in range(2):
                nc.tensor.matmul(
                    py[:isz, :D], _r(ot2[:, dc, i0:i0 + isz]), _r(wout_s[:, dc, :]),
                    start=(dc == 0), stop=(dc == 1))
            nc.scalar.activation(ysb[:isz, ic, :], py[:isz, :D],
                                 mybir.ActivationFunctionType.Copy)
            nc.sync.dma_start(out=y[t0 + i0:t0 + i0 + isz, :], in_=ysb[:isz, ic, :])


def _build():
    if "nc" in _CACHE:
        return _CACHE["nc"]
    nc = bacc.Bacc("TRN2", target_bir_lowering=False)
    xT = nc.dram_tensor("xT", [D, TOK], F32, kind="ExternalInput").ap()
    wqkvT = nc.dram_tensor("wqkvT", [D, 3 * D], F32, kind="ExternalInput").ap()
    woutT = nc.dram_tensor("woutT", [D, D], F32, kind="ExternalInput").ap()
    expBT = nc.dram_tensor("expBT", [N, H * N], BF16, kind="ExternalInput").ap()
    y = nc.dram_tensor("y", [TOK, D], F32, kind="ExternalOutput").ap()
    with tile.TileContext(nc) as tc, ExitStack() as ctx:
        _body(ctx, tc, xT, wqkvT, woutT, expBT, y)
    nc.compile()
    _CACHE["nc"] = nc
    return nc


def kernel(x, w_qkv, w_out, bias_table, rel_idx):
    x = np.asarray(x, dtype=np.float32)
    w_qkv = np.asarray(w_qkv, dtype=np.float32)
    w_out = np.asarray(w_out, dtype=np.float32)
    bias_table = np.asarray(bias_table, dtype=np.float32)
    rel_idx = np.asarray(rel_idx)

    # host-side layout prep
    # x[0]: [l, X, Y, w1, w2, d] -> xT [d, (X Y l w1 w2)]
    xt = np.ascontiguousarray(x[0].transpose(5, 1, 2, 0, 3, 4)).reshape(D, GX * GY * N)
    wqkvT = np.ascontiguousarray(w_qkv.T)
    woutT = np.ascontiguousarray(w_out.T)
    bias = bias_table[rel_idx]                  # [i, j, h]
    expBT = np.ascontiguousarray(
        np.exp(bias.astype(np.float64)).astype(np.float32).transpose(1, 2, 0)
    ).reshape(N, H * N).astype(ml_dtypes.bfloat16)

    nc = _build()
    in_maps = []
    for s in range(NCORES):
        xs = np.ascontiguousarray(xt[:, s * TOK:(s + 1) * TOK])
        in_maps.append({"xT": xs, "wqkvT": wqkvT, "woutT": woutT, "expBT": expBT})

    res = run_bass_kernel_spmd(nc, in_maps, core_ids=list(range(NCORES)), trace=TRACE)
    if TRACE and res.exec_time_ns is not None:
        print(f"HW exec time: {res.exec_time_ns} ns")
        _CACHE["exec_time_ns"] = res.exec_time_ns

    # gather: per-core y [9408, 256] -> [1, l, X, Y, w1, w2, d]
    out = np.empty((1, L, GX, GY, W, W, D), dtype=np.float32)
    for s in range(NCORES):
        yc = res.results[s]["y"].reshape(XPC, GY, L, W, W, D)
        out[0, :, s * XPC:(s + 1) * XPC] = yc.transpose(2, 0, 1, 3, 4, 5)
    return out


# revision 20
# speedup vs baseline: 1.2299x; 1.2299x over previous
"""Trainium2 Bass kernel for windowed 3D attention (sparse_attention).

Per window (256 windows on a 16x16 grid): N=294 tokens, d=256, 8 heads x 32.
qkv = x @ w_qkv.T ; A = softmax(q k^T/sqrt(dh) + bias) ; out = (A v) @ w_out.T

Sharding: data-parallel over the grid; core s takes X-rows [2s, 2s+2) = 32 windows.

Device strategy (best measured: ~787us HW exec, rel err 4.6e-3):
  - all matmul inputs bf16 (host-cast); 1/sqrt(dh) folded into Wq on host
  - Q^T/K^T d-major; V token-major; S^T[j,i] per head, row-packed (K=32) in
    head-groups of 3/3/2 over two alternating 3-bank PSUM pools so ScalarE
    exp streams one pool while PE fills the other
  - bias seeded into PSUM by identity matmuls (exact fp32 add), S accumulates
  - A^T = exp(S^T+B^T) straight from PSUM on ScalarE -> bf16 SBUF
  - rowsums via col-packed ones-matmuls -> PE-transpose -> reciprocal [i, head]
  - O^T = V^T A^T col-packed; PE-transpose -> normalize per (head,i) on GPSIMD
    (stride-0 free-dim broadcast of recip) -> PE-transpose back
  - Y = O_norm @ w_out^T token-major -> contiguous DMA out; host reassembles
"""

import os
from contextlib import ExitStack

import numpy as np
import ml_dtypes

import concourse.bass as bass
import concourse.mybir as mybir
import concourse.tile as tile
from concourse import bacc
from concourse.bass_utils import run_bass_kernel_spmd
from concourse.masks import make_identity

F32 = mybir.dt.float32
BF16 = mybir.dt.bfloat16

L, W, D, H = 6, 7, 256, 8
DH = D // H                      # 32
N = L * W * W                    # 294
GX = GY = 16
NCORES = 8
XPC = GX // NCORES               # X-rows per core
NW = XPC * GY                    # 32 windows per core
TOK = NW * N                     # 9408 tokens per core
SCALE = DH ** -0.5
NP = 384                         # N padded to 128 multiple

CH = [(0, 128), (128, 128), (256, 38)]    # j / i chunks

TRACE = False     # set by test.py for profiling runs
_CACHE = {}


def _bcast_free(ap, count):
    """Append a stride-0 innermost free dim of size `count` to an AP."""
    return bass.AP(tensor=ap.tensor, offset=ap.offset, ap=list(ap.ap) + [[0, count]])


def _body(ctx, tc, xT, wqkvT, woutT, biasT, y):
    nc = tc.nc

    const = ctx.enter_context(tc.tile_pool(name="const", bufs=1))
    xpool = ctx.enter_context(tc.tile_pool(name="xin", bufs=3))
    qkpool = ctx.enter_context(tc.tile_pool(name="qk", bufs=3))
    vpool = ctx.enter_context(tc.tile_pool(name="vtok", bufs=3))
    apool = ctx.enter_context(tc.tile_pool(name="at", bufs=5))
    opool = ctx.enter_context(tc.tile_pool(name="ot", bufs=3))
    onpool = ctx.enter_context(tc.tile_pool(name="onorm", bufs=3))
    o2pool = ctx.enter_context(tc.tile_pool(name="ot2", bufs=3))
    ypool = ctx.enter_context(tc.tile_pool(name="yout", bufs=2))
    rspool = ctx.enter_context(tc.tile_pool(name="rs", bufs=3))
    rcpool = ctx.enter_context(tc.tile_pool(name="recip", bufs=3))

    ps_a = ctx.enter_context(tc.tile_pool(name="ps_a", bufs=1, space="PSUM"))
    ps_b = ctx.enter_context(tc.tile_pool(name="ps_b", bufs=1, space="PSUM"))
    ps_m = ctx.enter_context(tc.tile_pool(name="ps_m", bufs=2, space="PSUM"))

    # ---- resident constants ----
    wqkv_s = const.tile([128, 2, 2 * D], BF16)     # Q^T,K^T weight cols (q pre-scaled)
    nc.sync.dma_start(out=wqkv_s, in_=wqkvT.rearrange("(c p) n -> p c n", c=2)[:, :, 0:2 * D])
    wv_s = const.tile([128, 2, D], BF16)
    nc.sync.dma_start(out=wv_s, in_=wqkvT.rearrange("(c p) n -> p c n", c=2)[:, :, 2 * D:3 * D])
    wout_s = const.tile([128, 2, D], BF16)
    nc.sync.dma_start(out=wout_s, in_=woutT.rearrange("(c p) n -> p c n", c=2))
    bias_s = const.tile([128, 3, H * N], BF16)
    for jc, (j0, jn) in enumerate(CH):
        nc.sync.dma_start(out=bias_s[:jn, jc, :], in_=biasT[j0:j0 + jn, :])
    ident = const.tile([128, 128], F32)
    make_identity(nc, ident)
    ident_b = const.tile([128, 128], BF16)
    make_identity(nc, ident_b)
    ones_b = const.tile([128, 1], BF16)
    nc.vector.memset(ones_b, 1.0)

    for w in range(NW):
        t0 = w * N
        # ---- load x window (d-major, bf16) ----
        xw = xpool.tile([128, 2, N], BF16)
        nc.sync.dma_start(out=xw, in_=xT.rearrange("(c p) t -> p c t", c=2)[:, :, t0:t0 + N])

        # ---- Q^T / K^T (d-major) ----
        qk = qkpool.tile([128, 6, N], BF16)      # 0,1: q^T (pre-scaled); 2,3: k^T; 4,5: -q^T
        for m in range(4):
            pq = ps_m.tile([128, 512], F32, tag="psmisc")
            for kc in range(2):
                nc.tensor.matmul(
                    pq[:, :N], wqkv_s[:, kc, m * 128:(m + 1) * 128], xw[:, kc, :],
                    start=(kc == 0), stop=(kc == 1))
            nc.vector.tensor_copy(qk[:, m, :], pq[:, :N])
        for g in range(2):
            nc.vector.tensor_scalar(qk[:, 4 + g, :], qk[:, g, :], -1.0, None,
                                    op0=mybir.AluOpType.mult)

        # ---- V token-major ----
        vtok = vpool.tile([128, 3, D], BF16)
        for jc, (j0, jn) in enumerate(CH):
            pv = ps_m.tile([128, 512], F32, tag="psmisc")
            for kc in range(2):
                nc.tensor.matmul(
                    pv[:jn, :D], xw[:, kc, j0:j0 + jn], wv_s[:, kc, :],
                    start=(kc == 0), stop=(kc == 1))
            nc.vector.tensor_copy(vtok[:jn, jc, :], pv[:jn, :D])

        # ---- S^T = bias (identity-seeded) + K^T.T @ Q^T ; exp -> A^T bf16 ----
        # head groups of 3/3/2 across two alternating 3-bank pools: ScalarE
        # streams exp on one pool while PE fills the other.
        at = []
        for jc, (j0, jn) in enumerate(CH):
            a_t = apool.tile([128, H * N], BF16, tag="at")
            for gi, heads in enumerate(([0, 1, 2], [3, 4, 5], [6, 7])):
                pool = ps_a if (3 * jc + gi) % 2 == 0 else ps_b
                ps3 = pool.tile([128, 3, 512], F32, tag="s3",
                                name=f"s3_{w}_{jc}_{gi}")
                for p, h in enumerate(heads):
                    nc.tensor.matmul(
                        ps3[:jn, p, :N], ident_b[:jn, :jn],
                        bias_s[:jn, jc, h * N:(h + 1) * N],
                        start=True, stop=False, skip_group_check=True)
                for p, h in enumerate(heads):
                    nc.tensor.matmul(
                        ps3[:jn, p, :N],
                        qk[32 * (h % 4):32 * (h % 4) + 32, 2 + h // 4, j0:j0 + jn],
                        qk[32 * (h % 4):32 * (h % 4) + 32, h // 4, :],
                        start=False, stop=True, tile_position=(32 * (h % 4), 0),
                        skip_group_check=True)
                ng = len(heads)
                nc.scalar.activation(
                    a_t[:jn, heads[0] * N:(heads[-1] + 1) * N].rearrange(
                        "p (c i) -> p c i", c=ng),
                    ps3[:jn, :ng, :N], mybir.ActivationFunctionType.Exp)
            at.append(a_t)

        # ---- rowsums (col-packed ones matmuls, accumulated over j chunks) ----
        prs = [ps_m.tile([128, 512], F32, tag="psmisc", name=f"prs{g}") for g in range(2)]
        for jc, (j0, jn) in enumerate(CH):
            for g in range(2):
                for c in range(4):
                    h = 4 * g + c
                    nc.tensor.matmul(
                        prs[g][32 * c:32 * c + 1, :N],
                        ones_b[:jn, :], at[jc][:jn, h * N:(h + 1) * N],
                        start=(jc == 0), stop=(jc == 2),
                        tile_position=(0, 32 * c), skip_group_check=True)
        rs_s = rspool.tile([128, 2, N], F32, tag="rcw", name=f"rcw{w}")
        for g in range(2):
            nc.vector.tensor_copy(rs_s[:, g, :], prs[g][:, :N])
        # transpose rowsums to [i, head] and take reciprocals
        rcp = rcpool.tile([128, 3, 8], F32, tag="rcp", name=f"rcp{w}")
        for ic, (i0, isz) in enumerate(CH):
            prt = ps_m.tile([128, 2, 128], F32, tag="psmisc", name=f"prt{w}_{ic}")
            for g in range(2):
                nc.tensor.transpose(prt[:isz, g, :], rs_s[:, g, i0:i0 + isz], ident)
            for g in range(2):
                srcp = prt[:isz, g, :].rearrange("p (c r) -> p c r", r=32)[:, :, 0]
                nc.vector.reciprocal(rcp[:isz, ic, 4 * g:4 * g + 4], srcp)

        # ---- O^T = V^T A^T (col-packed 4 heads) ----
        po = [ps_m.tile([128, 512], F32, tag="psmisc", name=f"po{w}_{g}") for g in range(2)]
        for jc, (j0, jn) in enumerate(CH):
            for g in range(2):
                for c in range(4):
                    h = 4 * g + c
                    nc.tensor.matmul(
                        po[g][32 * c:32 * c + 32, :N],
                        vtok[:jn, jc, 32 * h:32 * h + 32],
                        at[jc][:jn, h * N:(h + 1) * N],
                        start=(jc == 0), stop=(jc == 2),
                        tile_position=(0, 32 * c), skip_group_check=True)
        ot = opool.tile([128, 2, NP], BF16, tag="oraw", name=f"orw{w}")
        for g in range(2):
            nc.vector.tensor_copy(ot[:, g, :N], po[g][:, :N])

        # ---- PE transpose O^T -> O (bf16), normalize, transpose back ----
        onrm = []
        for ic, (i0, isz) in enumerate(CH):
            ptr = ps_m.tile([128, 2, 128], BF16, tag="psmisc", name=f"ptr{w}_{ic}")
            for g in range(2):
                nc.tensor.transpose(ptr[:isz, g, :], ot[:, g, i0:i0 + isz], ident_b)
            onr = onpool.tile([128, 2, 128], BF16, tag="onr", name=f"onr{w}_{ic}")
            nc.vector.tensor_copy(onr[:isz, :, :], ptr[:isz, :, :])
            onm = onpool.tile([128, 2, 128], BF16, tag="onm", name=f"onm{w}_{ic}")
            nc.gpsimd.tensor_tensor(
                out=onm[:isz, :, :].rearrange("p g e -> p (g e)").rearrange("p (h e) -> p h e", h=8),
                in0=onr[:isz, :, :].rearrange("p g e -> p (g e)").rearrange("p (h e) -> p h e", h=8),
                in1=_bcast_free(rcp[:isz, ic, :], 32),
                op=mybir.AluOpType.mult)
            onrm.append(onm)

        ot2 = o2pool.tile([128, 2, NP], BF16, tag="ot2", name=f"ot2_{w}")
        for ic, (i0, isz) in enumerate(CH):
            pt2 = ps_m.tile([128, 2, 128], BF16, tag="psmisc", name=f"pt2_{w}_{ic}")
            for dc in range(2):
                nc.tensor.transpose(pt2[:, dc, :isz], onrm[ic][:isz, dc, :], ident_b[:isz, :isz])
            nc.vector.tensor_copy(ot2[:, :, i0:i0 + isz], pt2[:, :, :isz])

        # ---- Y = O_norm @ w_out^T (token-major) + store ----
        ysb = ypool.tile([128, 3, D], F32)
        for ic, (i0, isz) in enumerate(CH):
            py = ps_m.tile([128, 512], F32, tag="psmisc")
            for dc in range(2):
                nc.tensor.matmul(
                    py[:isz, :D], ot2[:, dc, i0:i0 + isz], wout_s[:, dc, :],
                    start=(dc == 0), stop=(dc == 1))
            nc.vector.tensor_copy(ysb[:isz, ic, :], py[:isz, :D])
            nc.scalar.dma_start(out=y[t0 + i0:t0 + i0 + isz, :], in_=ysb[:isz, ic, :])


# revision 21
# speedup vs baseline: 1.2312x; 1.0011x over previous
"""Trainium2 Bass kernel for windowed 3D attention (sparse_attention).

Per window (256 windows on a 16x16 grid): N=294 tokens, d=256, 8 heads x 32.
qkv = x @ w_qkv.T ; A = softmax(q k^T/sqrt(dh) + bias) ; out = (A v) @ w_out.T

Sharding: data-parallel over the grid; core s takes X-rows [2s, 2s+2) = 32 windows.

Device strategy (best measured: ~787us HW exec, rel err 4.6e-3):
  - all matmul inputs bf16 (host-cast); 1/sqrt(dh) folded into Wq on host
  - Q^T/K^T d-major; V token-major; S^T[j,i] per head, row-packed (K=32) in
    head-groups of 3/3/2 over two alternating 3-bank PSUM pools so ScalarE
    exp streams one pool while PE fills the other
  - bias seeded into PSUM by identity matmuls (exact fp32 add), S accumulates
  - A^T = exp(S^T+B^T) straight from PSUM on ScalarE -> bf16 SBUF
  - rowsums via col-packed ones-matmuls -> PE-transpose -> reciprocal [i, head]
  - O^T = V^T A^T col-packed; PE-transpose -> normalize per (head,i) on GPSIMD
    (stride-0 free-dim broadcast of recip) -> PE-transpose back
  - Y = O_norm @ w_out^T token-major -> contiguous DMA out; host reassembles
"""

import os
from contextlib import ExitStack

import numpy as np
import ml_dtypes

import concourse.bass as bass
import concourse.mybir as mybir
import concourse.tile as tile
from concourse import bacc
from concourse.bass_utils import run_bass_kernel_spmd
from concourse.masks import make_identity

F32 = mybir.dt.float32
BF16 = mybir.dt.bfloat16

L, W, D, H = 6, 7, 256, 8
DH = D // H                      # 32
N = L * W * W                    # 294
GX = GY = 16
NCORES = 8
XPC = GX // NCORES               # X-rows per core
NW = XPC * GY                    # 32 windows per core
TOK = NW * N                     # 9408 tokens per core
SCALE = DH ** -0.5
NP = 384                         # N padded to 128 multiple

CH = [(0, 128), (128, 128), (256, 38)]    # j / i chunks

TRACE = False     # set by test.py for profiling runs
_CACHE = {}


def _bcast_free(ap, count):
    """Append a stride-0 innermost free dim of size `count` to an AP."""
    return bass.AP(tensor=ap.tensor, offset=ap.offset, ap=list(ap.ap) + [[0, count]])


def _body(ctx, tc, xT, wqkvT, woutT, biasT, y):
    nc = tc.nc

    const = ctx.enter_context(tc.tile_pool(name="const", bufs=1))
    xpool = ctx.enter_context(tc.tile_pool(name="xin", bufs=3))
    qkpool = ctx.enter_context(tc.tile_pool(name="qk", bufs=3))
    vpool = ctx.enter_context(tc.tile_pool(name="vtok", bufs=3))
    apool = ctx.enter_context(tc.tile_pool(name="at", bufs=5))
    opool = ctx.enter_context(tc.tile_pool(name="ot", bufs=3))
    onpool = ctx.enter_context(tc.tile_pool(name="onorm", bufs=3))
    o2pool = ctx.enter_context(tc.tile_pool(name="ot2", bufs=3))
    ypool = ctx.enter_context(tc.tile_pool(name="yout", bufs=2))
    rspool = ctx.enter_context(tc.tile_pool(name="rs", bufs=3))
    rcpool = ctx.enter_context(tc.tile_pool(name="recip", bufs=3))

    ps_a = ctx.enter_context(tc.tile_pool(name="ps_a", bufs=1, space="PSUM"))
    ps_b = ctx.enter_context(tc.tile_pool(name="ps_b", bufs=1, space="PSUM"))
    ps_c = ctx.enter_context(tc.tile_pool(name="ps_c", bufs=1, space="PSUM"))
    ps_m = ctx.enter_context(tc.tile_pool(name="ps_m", bufs=2, space="PSUM"))

    # ---- resident constants ----
    wqkv_s = const.tile([128, 2, 2 * D], BF16)     # Q^T,K^T weight cols (q pre-scaled)
    nc.sync.dma_start(out=wqkv_s, in_=wqkvT.rearrange("(c p) n -> p c n", c=2)[:, :, 0:2 * D])
    wv_s = const.tile([128, 2, D], BF16)
    nc.sync.dma_start(out=wv_s, in_=wqkvT.rearrange("(c p) n -> p c n", c=2)[:, :, 2 * D:3 * D])
    wout_s = const.tile([128, 2, D], BF16)
    nc.sync.dma_start(out=wout_s, in_=woutT.rearrange("(c p) n -> p c n", c=2))
    bias_s = const.tile([128, 3, H * N], BF16)
    for jc, (j0, jn) in enumerate(CH):
        nc.sync.dma_start(out=bias_s[:jn, jc, :], in_=biasT[j0:j0 + jn, :])
    ident = const.tile([128, 128], F32)
    make_identity(nc, ident)
    ident_b = const.tile([128, 128], BF16)
    make_identity(nc, ident_b)
    ones_b = const.tile([128, 1], BF16)
    nc.vector.memset(ones_b, 1.0)

    for w in range(NW):
        t0 = w * N
        # ---- load x window (d-major, bf16) ----
        xw = xpool.tile([128, 2, N], BF16)
        nc.sync.dma_start(out=xw, in_=xT.rearrange("(c p) t -> p c t", c=2)[:, :, t0:t0 + N])

        # ---- Q^T / K^T (d-major) ----
        qk = qkpool.tile([128, 6, N], BF16)      # 0,1: q^T (pre-scaled); 2,3: k^T; 4,5: -q^T
        for m in range(4):
            pq = ps_m.tile([128, 512], F32, tag="psmisc")
            for kc in range(2):
                nc.tensor.matmul(
                    pq[:, :N], wqkv_s[:, kc, m * 128:(m + 1) * 128], xw[:, kc, :],
                    start=(kc == 0), stop=(kc == 1))
            nc.vector.tensor_copy(qk[:, m, :], pq[:, :N])
        for g in range(2):
            nc.vector.tensor_scalar(qk[:, 4 + g, :], qk[:, g, :], -1.0, None,
                                    op0=mybir.AluOpType.mult)

        # ---- V token-major ----
        vtok = vpool.tile([128, 3, D], BF16)
        for jc, (j0, jn) in enumerate(CH):
            pv = ps_m.tile([128, 512], F32, tag="psmisc")
            for kc in range(2):
                nc.tensor.matmul(
                    pv[:jn, :D], xw[:, kc, j0:j0 + jn], wv_s[:, kc, :],
                    start=(kc == 0), stop=(kc == 1))
            nc.vector.tensor_copy(vtok[:jn, jc, :], pv[:jn, :D])

        # ---- S^T = bias (identity-seeded) + K^T.T @ Q^T ; exp -> A^T bf16 ----
        # head groups of 3/3/2 across two alternating 3-bank pools: ScalarE
        # streams exp on one pool while PE fills the other.
        at = []
        for jc, (j0, jn) in enumerate(CH):
            a_t = apool.tile([128, H * N], BF16, tag="at")
            for gi, heads in enumerate(([0, 1], [2, 3], [4, 5], [6, 7])):
                pool = (ps_a, ps_b, ps_c)[(4 * jc + gi) % 3]
                ps3 = pool.tile([128, 2, 512], F32, tag="s2",
                                name=f"s2_{w}_{jc}_{gi}")
                ng = len(heads)
                for p, h in enumerate(heads):
                    nc.tensor.matmul(
                        ps3[:jn, p, :N], ident_b[:jn, :jn],
                        bias_s[:jn, jc, h * N:(h + 1) * N],
                        start=True, stop=False, skip_group_check=True)
                for p, h in enumerate(heads):
                    nc.tensor.matmul(
                        ps3[:jn, p, :N],
                        qk[32 * (h % 4):32 * (h % 4) + 32, 2 + h // 4, j0:j0 + jn],
                        qk[32 * (h % 4):32 * (h % 4) + 32, h // 4, :],
                        start=False, stop=True, tile_position=(32 * (h % 4), 0),
                        skip_group_check=True)
                nc.scalar.activation(
                    a_t[:jn, heads[0] * N:(heads[-1] + 1) * N].rearrange(
                        "p (c i) -> p c i", c=ng),
                    ps3[:jn, :ng, :N], mybir.ActivationFunctionType.Exp)
            at.append(a_t)

        # ---- rowsums (col-packed ones matmuls, accumulated over j chunks) ----
        prs = [ps_m.tile([128, 512], F32, tag="psmisc", name=f"prs{g}") for g in range(2)]
        for jc, (j0, jn) in enumerate(CH):
            for g in range(2):
                for c in range(4):
                    h = 4 * g + c
                    nc.tensor.matmul(
                        prs[g][32 * c:32 * c + 1, :N],
                        ones_b[:jn, :], at[jc][:jn, h * N:(h + 1) * N],
                        start=(jc == 0), stop=(jc == 2),
                        tile_position=(0, 32 * c), skip_group_check=True)
        rs_s = rspool.tile([128, 2, N], F32, tag="rcw", name=f"rcw{w}")
        for g in range(2):
            nc.vector.tensor_copy(rs_s[:, g, :], prs[g][:, :N])
        # transpose rowsums to [i, head] and take reciprocals
        rcp = rcpool.tile([128, 3, 8], F32, tag="rcp", name=f"rcp{w}")
        for ic, (i0, isz) in enumerate(CH):
            prt = ps_m.tile([128, 2, 128], F32, tag="psmisc", name=f"prt{w}_{ic}")
            for g in range(2):
                nc.tensor.transpose(prt[:isz, g, :], rs_s[:, g, i0:i0 + isz], ident)
            for g in range(2):
                srcp = prt[:isz, g, :].rearrange("p (c r) -> p c r", r=32)[:, :, 0]
                nc.vector.reciprocal(rcp[:isz, ic, 4 * g:4 * g + 4], srcp)

        # ---- O^T = V^T A^T (col-packed 4 heads) ----
        po = [ps_m.tile([128, 512], F32, tag="psmisc", name=f"po{w}_{g}") for g in range(2)]
        for jc, (j0, jn) in enumerate(CH):
            for g in range(2):
                for c in range(4):
                    h = 4 * g + c
                    nc.tensor.matmul(
                        po[g][32 * c:32 * c + 32, :N],
                        vtok[:jn, jc, 32 * h:32 * h + 32],
                        at[jc][:jn, h * N:(h + 1) * N],
                        start=(jc == 0), stop=(jc == 2),
                        tile_position=(0, 32 * c), skip_group_check=True)
        ot = opool.tile([128, 2, NP], BF16, tag="oraw", name=f"orw{w}")
        for g in range(2):
            nc.vector.tensor_copy(ot[:, g, :N], po[g][:, :N])

        # ---- PE transpose O^T -> O (bf16), normalize, transpose back ----
        onrm = []
        for ic, (i0, isz) in enumerate(CH):
            ptr = ps_m.tile([128, 2, 128], BF16, tag="psmisc", name=f"ptr{w}_{ic}")
            for g in range(2):
                nc.tensor.transpose(ptr[:isz, g, :], ot[:, g, i0:i0 + isz], ident_b)
            onr = onpool.tile([128, 2, 128], BF16, tag="onr", name=f"onr{w}_{ic}")
            nc.vector.tensor_copy(onr[:isz, :, :], ptr[:isz, :, :])
            onm = onpool.tile([128, 2, 128], BF16, tag="onm", name=f"onm{w}_{ic}")
            nc.gpsimd.tensor_tensor(
                out=onm[:isz, :, :].rearrange("p g e -> p (g e)").rearrange("p (h e) -> p h e", h=8),
                in0=onr[:isz, :, :].rearrange("p g e -> p (g e)").rearrange("p (h e) -> p h e", h=8),
                in1=_bcast_free(rcp[:isz, ic, :], 32),
                op=mybir.AluOpType.mult)
            onrm.append(onm)

        ot2 = o2pool.tile([128, 2, NP], BF16, tag="ot2", name=f"ot2_{w}")
        for ic, (i0, isz) in enumerate(CH):
            pt2 = ps_m.tile([128, 2, 128], BF16, tag="psmisc", name=f"pt2_{w}_{ic}")
            for dc in range(2):
                nc.tensor.transpose(pt2[:, dc, :isz], onrm[ic][:isz, dc, :], ident_b[:isz, :isz])
            nc.vector.tensor_copy(ot2[:, :, i0:i0 + isz], pt2[:, :, :isz])

        # ---- Y = O_norm @ w_out^T (token-major) + store ----
        ysb = ypool.tile([128, 3, D], F32)
        for ic, (i0, isz) in enumerate(CH):
            py = ps_m.tile([128, 512], F32, tag="psmisc")
            for dc in range(2):
                nc.tensor.matmul(
                    py[:isz, :D], ot2[:, dc, i0:i0 + isz], wout_s[:, dc, :],
                    start=(dc == 0), stop=(dc == 1))
            nc.vector.tensor_copy(ysb[:isz, ic, :], py[:isz, :D])
            nc.scalar.dma_start(out=y[t0 + i0:t0 + i0 + isz, :], in_=ysb[:isz, ic, :])
